# revision 11
# baseline (speedup 1.0000x reference)
"""Cached Mistral self-attention (prefill) on 8 Trainium2 NeuronCores.

Sharding: tensor-parallel over heads. Core c owns query heads 4c..4c+3
(rows 512c:512(c+1) of w_q / w_o-columns... see below) and KV head c
(rows 128c:128(c+1) of w_k / w_v).

Per-core dataflow (all matmul data fp16, fp32 PSUM accumulation):
  phase 1: xT tiles via XBAR DMA-transpose; qT/kT/vT projections with
           W^T stationary; RoPE fused into the PSUM->SBUF evacuation.
  phase 2: transposed-scores flash attention: S^T = kT.T @ qT chunks,
           exp(S - 4) on ScalarE straight into SBUF as P^T, causal
           masking by 0/1 mask multiply, row-sums via ones-matmul,
           O^T accumulation with natural-layout V, normalization by
           broadcast reciprocal row-sums.
  phase 3: o^T slices AllGather'd across cores per 512-token chunk
           (collective overlaps later attention chunks); o_proj
           y[:, 512c:512(c+1)] = o_full @ w_o[512c:512(c+1), :].T.
Host: shard/cast inputs, concat y column slices.
"""
import sys

sys.path.insert(0, "/opt/trn_rl_repo")

import numpy as np

import concourse.bass as bass
import concourse.mybir as mybir
import concourse.tile as tile
from concourse.bass_utils import run_bass_kernel_spmd

N_CORES = 8
T, H, D = 2048, 32, 128
INNER = H * D          # 4096
HL = H // N_CORES      # 4 local q heads
DQ = HL * D            # 512
NF = INNER // 128      # 32 contraction tiles
NTT = T // 128         # 16 token tiles
NG = 4                 # 512-token chunks
CH = T // NG           # 512
EXP_BIAS = -4.0
ROPE_BASE = 10000.0

f16 = mybir.dt.float16
f32 = mybir.dt.float32

_PROGRAM_CACHE = {}


def _split_excess_waits(nc, limit=1):
    """walrus in this toolchain rejects >1 sync-wait per instruction; move
    extra waits onto NOPs inserted just before the offending instruction."""
    for f in nc.m.functions:
        for bb in f.blocks:
            insts = bb.instructions
            new_list = []
            changed = False
            for inst in insts:
                si = inst.sync_info
                if si is not None and si.on_wait and len(si.on_wait) > limit:
                    waits = list(si.on_wait)
                    extra, keep = waits[:-limit], waits[-limit:]
                    k = 0
                    while extra:
                        chunk, extra = extra[:limit], extra[limit:]
                        new_list.append(mybir.InstNoOp(
                            name=f"{inst.name}-waitsplit{k}",
                            sync_info=mybir.SyncInfo(on_wait=chunk, on_update=[]),
                            bass_nofuse=True, engine=inst.engine))
                        k += 1
                    si.on_wait = keep
                    inst.sync_info = si
                    changed = True
                new_list.append(inst)
            if changed:
                bb.instructions = new_list


def _build(debug=False):
    nc = bass.Bass(num_devices=N_CORES)

    x16 = nc.dram_tensor("x16", [T, INNER], f16, kind="ExternalInput")
    wq16 = nc.dram_tensor("wq16", [DQ, INNER], f16, kind="ExternalInput")
    wk16 = nc.dram_tensor("wk16", [D, INNER], f16, kind="ExternalInput")
    wv16 = nc.dram_tensor("wv16", [D, INNER], f16, kind="ExternalInput")
    wo16 = nc.dram_tensor("wo16", [DQ, INNER], f16, kind="ExternalInput")
    cosq = nc.dram_tensor("cosq", [D, T], f16, kind="ExternalInput")
    sinq = nc.dram_tensor("sinq", [D, T], f16, kind="ExternalInput")
    cosk = nc.dram_tensor("cosk", [D, T], f16, kind="ExternalInput")
    sink = nc.dram_tensor("sink", [D, T], f16, kind="ExternalInput")
    masks = nc.dram_tensor("masks", [4, 128, CH], f16, kind="ExternalInput")
    ones_col = nc.dram_tensor("ones_col", [128, 1], f16, kind="ExternalInput")
    ones_row = nc.dram_tensor("ones_row", [1, 128], f16, kind="ExternalInput")
    ident = nc.dram_tensor("ident", [128, 128], f16, kind="ExternalInput")

    y_out = nc.dram_tensor("y", [T, DQ], f32, kind="ExternalOutput")
    dbg = {}
    if debug:
        dbg["qT"] = nc.dram_tensor("dbg_qT", [HL, D, T], f32, kind="ExternalOutput")
        dbg["kT"] = nc.dram_tensor("dbg_kT", [D, T], f32, kind="ExternalOutput")
        dbg["v"] = nc.dram_tensor("dbg_v", [T, D], f32, kind="ExternalOutput")
        dbg["oT"] = nc.dram_tensor("dbg_oT", [DQ, T], f32, kind="ExternalOutput")

    with tile.TileContext(nc) as tc:
        with tc.tile_pool(name="persist", bufs=1) as pp, \
             tc.tile_pool(name="dramp", bufs=1, space="DRAM") as dramp:
            o_send = [dramp.tile([DQ, CH], f16, name=f"o_send{g}") for g in range(NG)]
            o_gath = [dramp.tile([INNER, CH], f16, addr_space="Shared",
                                 name=f"o_gath{g}") for g in range(NG)]
            # ---- resident tensors -------------------------------------
            wkT = pp.tile([128, NF, D], f16, name="wkT")
            wvT = pp.tile([128, NF, D], f16, name="wvT")
            for fi in range(NF):
                fs = slice(fi * 128, (fi + 1) * 128)
                nc.sync.dma_start_transpose(wkT[:, fi], wk16[:, fs])
                nc.sync.dma_start_transpose(wvT[:, fi], wv16[:, fs])
            cq = pp.tile([128, T], f16, name="cq")
            sq = pp.tile([128, T], f16, name="sq")
            ck = pp.tile([128, T], f16, name="ck")
            sk = pp.tile([128, T], f16, name="sk")
            nc.sync.dma_start(cq[:], cosq[:])
            nc.sync.dma_start(sq[:], sinq[:])
            nc.sync.dma_start(ck[:], cosk[:])
            nc.sync.dma_start(sk[:], sink[:])
            msk = pp.tile([128, 4, CH], f16, name="msk")
            nc.sync.dma_start(msk[:], masks.rearrange("r p c -> p r c"))
            onc = pp.tile([128, 1], f16, name="onc")
            onr = pp.tile([1, 128], f16, name="onr")
            idn = pp.tile([128, 128], f16, name="idn")
            nc.sync.dma_start(onc[:], ones_col[:])
            nc.sync.dma_start(onr[:], ones_row[:])
            nc.sync.dma_start(idn[:], ident[:])
            expb = pp.tile([128, 1], f32, name="expb")
            nc.vector.memset(expb[:], EXP_BIAS)

            qT = pp.tile([128, HL, T], f16, name="qT")   # per head [d, t]
            kT = pp.tile([128, T], f16, name="kT")
            vn = pp.tile([128, NTT, D], f16, name="vn")  # v natural [t-tile, d]

            # ---- phase 1: QKV projections + rope ----------------------
            with tc.tile_pool(name="p1ps", bufs=1, space="PSUM") as p1ps, \
                 tc.tile_pool(name="p1sb", bufs=4) as p1sb, \
                 tc.tile_pool(name="p1wq", bufs=1) as p1wq, \
                 tc.tile_pool(name="p1tr", bufs=2, space="PSUM") as p1tr:
                wqT = p1wq.tile([128, NF, DQ], f16, name="wqT")
                for fi in range(NF):
                    nc.sync.dma_start_transpose(
                        wqT[:, fi], wq16[:, fi * 128:(fi + 1) * 128])
                for g in range(NG):
                    tsl = slice(g * CH, (g + 1) * CH)
                    qps = [p1ps.tile([128, CH], f32, name=f"qps{d}") for d in range(HL)]
                    kps = p1ps.tile([128, CH], f32, name="kps")
                    vps = p1ps.tile([128, CH], f32, name="vps")
                    for fi in range(NF):
                        xT = p1sb.tile([128, CH], f16, name="xT")
                        nc.sync.dma_start_transpose(
                            xT[:], x16[tsl, fi * 128:(fi + 1) * 128])
                        st, sp = fi == 0, fi == NF - 1
                        for d in range(HL):
                            nc.tensor.matmul(qps[d][:], wqT[:, fi, d * 128:(d + 1) * 128],
                                             xT[:], start=st, stop=sp)
                        nc.tensor.matmul(kps[:], wkT[:, fi], xT[:], start=st, stop=sp)
                        nc.tensor.matmul(vps[:], wvT[:, fi], xT[:], start=st, stop=sp)
                    # rope evac: out = psum*cos + shift(psum)*sin
                    for d in range(HL):
                        t1 = p1sb.tile([128, CH], f16, name="t1")
                        t2 = p1sb.tile([128, CH], f16, name="t2")
                        nc.vector.tensor_tensor(t1[:], qps[d][:], cq[:, tsl],
                                                mybir.AluOpType.mult)
                        nc.vector.tensor_tensor(t2[0:64], qps[d][64:128], sq[0:64, tsl],
                                                mybir.AluOpType.mult)
                        nc.vector.tensor_tensor(t2[64:128], qps[d][0:64], sq[64:128, tsl],
                                                mybir.AluOpType.mult)
                        nc.vector.tensor_tensor(qT[:, d, tsl], t1[:], t2[:],
                                                mybir.AluOpType.add)
                    t1 = p1sb.tile([128, CH], f16, name="t1")
                    t2 = p1sb.tile([128, CH], f16, name="t2")
                    nc.vector.tensor_tensor(t1[:], kps[:], ck[:, tsl],
                                            mybir.AluOpType.mult)
                    nc.vector.tensor_tensor(t2[0:64], kps[64:128], sk[0:64, tsl],
                                            mybir.AluOpType.mult)
                    nc.vector.tensor_tensor(t2[64:128], kps[0:64], sk[64:128, tsl],
                                            mybir.AluOpType.mult)
                    nc.vector.tensor_tensor(kT[:, tsl], t1[:], t2[:],
                                            mybir.AluOpType.add)
                    # v: evac vT then PE-transpose to natural layout
                    vt = p1sb.tile([128, CH], f16, name="vt")
                    nc.vector.tensor_copy(vt[:], vps[:])
                    for tt in range(4):
                        vtr = p1tr.tile([128, 128], f16, name="vtr")
                        nc.tensor.transpose(vtr[:], vt[:, tt * 128:(tt + 1) * 128], idn[:])
                        nc.scalar.copy(vn[:, g * 4 + tt], vtr[:])

            if debug:
                for d in range(HL):
                    for tt in range(NTT):
                        db = pp.tile([128, 128], f32, name="dbgq", tag="dbgq")
                        nc.vector.tensor_copy(db[:], qT[:, d, tt * 128:(tt + 1) * 128])
                        nc.sync.dma_start(dbg["qT"][d, :, tt * 128:(tt + 1) * 128], db[:])
                dbk = pp.tile([128, T], f32, name="dbgk")
                nc.vector.tensor_copy(dbk[:], kT[:])
                nc.sync.dma_start(dbg["kT"][:], dbk[:])
                dbv = pp.tile([128, NTT, D], f32, name="dbgv")
                nc.vector.tensor_copy(dbv[:], vn[:])
                nc.sync.dma_start(dbg["v"].rearrange("(n p) d -> p n d", p=128), dbv[:])

            # ---- phases 2+3 -------------------------------------------
            with tc.tile_pool(name="p2S", bufs=2, space="PSUM") as p2S, \
                 tc.tile_pool(name="p2O", bufs=2, space="PSUM") as p2O, \
                 tc.tile_pool(name="p2s", bufs=1, space="PSUM") as p2s, \
                 tc.tile_pool(name="p2b", bufs=1, space="PSUM") as p2b, \
                 tc.tile_pool(name="p3y", bufs=2, space="PSUM") as p3y, \
                 tc.tile_pool(name="p2sb", bufs=6) as p2sb, \
                 tc.tile_pool(name="p2m", bufs=2) as p2m, \
                 tc.tile_pool(name="p2o", bufs=2) as p2o, \
                 tc.tile_pool(name="p3w", bufs=1) as p3w, \
                 tc.tile_pool(name="p3sb", bufs=2) as p3sb:

                woT = p3w.tile([128, NF, DQ], f16, name="woT")
                for fi in range(NF):
                    nc.sync.dma_start_transpose(
                        woT[:, fi], wo16[:, fi * 128:(fi + 1) * 128])

                def attention_chunk(g):
                    nt = 4 * (g + 1)          # tk tiles touched
                    tqs = slice(g * CH, (g + 1) * CH)
                    och = p2o.tile([128, HL, CH], f16, name="och")
                    for h in range(HL):
                        ops = p2O.tile([128, CH], f32, name="ops")
                        sps = p2s.tile([1, CH], f32, name="sps")
                        for j in range(nt):
                            Sps = p2S.tile([128, CH], f32, name="Sps")
                            nc.tensor.matmul(Sps[:], kT[:, j * 128:(j + 1) * 128],
                                             qT[:, h, tqs], start=True, stop=True)
                            PT = p2sb.tile([128, CH], f16, name="PT")
                            nc.scalar.activation(PT[:], Sps[:],
                                                 mybir.ActivationFunctionType.Exp,
                                                 bias=expb[:], scale=1.0)
                            r = j - 4 * g
                            if r >= 0:
                                nc.vector.tensor_tensor(PT[:], PT[:], msk[:, r],
                                                        mybir.AluOpType.mult)
                            st, sp = j == 0, j == nt - 1
                            nc.tensor.matmul(sps[:], onc[:], PT[:], start=st, stop=sp)
                            nc.tensor.matmul(ops[:], vn[:, j], PT[:], start=st, stop=sp)
                        s16 = p2m.tile([1, CH], f16, name="s16")
                        nc.scalar.copy(s16[:], sps[:])
                        bps = p2b.tile([128, CH], f32, name="bps")
                        nc.tensor.matmul(bps[:], onr[:], s16[:], start=True, stop=True)
                        rs = p2m.tile([128, CH], f32, name="rs")
                        nc.vector.reciprocal(rs[:], bps[:])
                        nc.vector.tensor_tensor(och[:, h], ops[:], rs[:],
                                                mybir.AluOpType.mult)
                    nc.sync.dma_start(
                        o_send[g].rearrange("(h d) t -> d h t", d=128), och[:])
                    nc.gpsimd.collective_compute(
                        "AllGather", mybir.AluOpType.bypass,
                        replica_groups=[list(range(N_CORES))],
                        ins=[o_send[g][:]], outs=[o_gath[g][:]])
                    if debug:
                        dbo = pp.tile([128, HL, CH], f32, name="dbgo")
                        nc.vector.tensor_copy(dbo[:], och[:])
                        nc.sync.dma_start(
                            dbg["oT"].rearrange("(h d) t -> d h t", d=128)[:, :, tqs],
                            dbo[:])

                def oproj_chunk(g):
                    ogb = p3sb.tile([128, NF, CH], f16, name="ogb")
                    for m in range(NF):
                        nc.sync.dma_start(ogb[:, m], o_gath[g][m * 128:(m + 1) * 128, :])
                    for tt in range(4):
                        yps = p3y.tile([128, DQ], f32, name="yps")
                        for m in range(NF):
                            nc.tensor.matmul(yps[:], ogb[:, m, tt * 128:(tt + 1) * 128],
                                             woT[:, m], start=(m == 0), stop=(m == NF - 1))
                        ysb = p3sb.tile([128, DQ], f32, name="ysb")
                        nc.vector.tensor_copy(ysb[:], yps[:])
                        rows = slice(g * CH + tt * 128, g * CH + (tt + 1) * 128)
                        nc.sync.dma_start(y_out[rows, :], ysb[:])

                attention_chunk(0)
                attention_chunk(1)
                oproj_chunk(0)
                attention_chunk(2)
                oproj_chunk(1)
                attention_chunk(3)
                oproj_chunk(2)
                oproj_chunk(3)

    _split_excess_waits(nc)
    return nc


def _host_consts():
    inv = 1.0 / (ROPE_BASE ** (np.arange(0, D, 2, dtype=np.float64) / D))
    tpos = np.arange(T, dtype=np.float64)
    freqs = np.outer(tpos, inv)                       # [T, D/2]
    emb = np.concatenate([freqs, freqs], axis=-1)     # [T, D]
    cos = np.cos(emb).T                               # [D, T]
    sin = np.sin(emb).T
    # sign-folded sin for the qT-layout rotation
    sinf = sin.copy()
    sinf[:64] = -sin[:64]
    scale = 1.0 / np.sqrt(D)
    cosq = (cos * scale).astype(np.float16)
    sinq = (sinf * scale).astype(np.float16)
    cosk = cos.astype(np.float16)
    sink = sinf.astype(np.float16)
    # masks[r][tk, tq] for the diagonal 4-tile group; block i' = tq//128:
    # i' < r -> 0 ; i' == r -> (tk <= tq) ; i' > r -> 1
    m = np.zeros((4, 128, CH), np.float16)
    tk = np.arange(128)[:, None]
    for r in range(4):
        for ip in range(4):
            blk = slice(ip * 128, (ip + 1) * 128)
            if ip < r:
                m[r, :, blk] = 0.0
            elif ip == r:
                m[r, :, blk] = (tk <= np.arange(128)[None, :]).astype(np.float16)
            else:
                m[r, :, blk] = 1.0
    return {
        "cosq": cosq, "sinq": sinq, "cosk": cosk, "sink": sink, "masks": m,
        "ones_col": np.ones((128, 1), np.float16),
        "ones_row": np.ones((1, 128), np.float16),
        "ident": np.eye(128, dtype=np.float16),
    }


def make_in_maps(stm, w_q, w_k, w_v, w_o):
    x16 = np.ascontiguousarray(stm.reshape(T, INNER).astype(np.float16))
    consts = _host_consts()
    wq = w_q.astype(np.float16)
    wk = w_k.astype(np.float16)
    wv = w_v.astype(np.float16)
    wo = w_o.astype(np.float16)
    in_maps = []
    for c in range(N_CORES):
        qs = slice(c * DQ, (c + 1) * DQ)
        ks = slice(c * D, (c + 1) * D)
        in_maps.append({
            "x16": x16,
            "wq16": np.ascontiguousarray(wq[qs]),
            "wk16": np.ascontiguousarray(wk[ks]),
            "wv16": np.ascontiguousarray(wv[ks]),
            "wo16": np.ascontiguousarray(wo[qs]),
            **consts,
        })
    return in_maps


def kernel(stm, w_q, w_k, w_v, w_o):
    key = "prog"
    if key not in _PROGRAM_CACHE:
        _PROGRAM_CACHE[key] = _build(debug=False)
    nc = _PROGRAM_CACHE[key]
    in_maps = make_in_maps(stm, w_q, w_k, w_v, w_o)
    res = run_bass_kernel_spmd(nc, in_maps, list(range(N_CORES)))
    y = np.concatenate([res.results[c]["y"] for c in range(N_CORES)], axis=1)
    return y.reshape(stm.shape).astype(np.float32)
